# revision 3
# baseline (speedup 1.0000x reference)
"""Trainium2 Bass kernel for nn_AvgPoolVectorsPerWSI (segment-mean over groups).

Math: x [N=2048, M=512, 7, 7], idx [N] in [0,64)
  out[g, m] = mean over {n: idx[n]==g} and spatial of x[n, m, :, :]  -> [64, 512, 1, 1]

Strategy (no collectives needed):
  - Shard over M: core k handles an m-slice of 64 channels, reading its x
    slice [2048, 64, 49] (25.7 MB) exactly once -> memory-bound.
  - The HBM stream is executed by 16 SDMA engines; each engine serves a
    fixed set of 8 SBUF partitions (engine 2j: partitions {4j..4j+3,
    32+4j..32+4j+3}; engine 2j+1: {64+4j.., 96+4j..}) at ~469 ns per
    12.5 KB row-descriptor.  Engine 15 (partitions 92-95 / 124-127) is
    empirically ~20-25% slower in many periods, and with a uniform
    16x128 layout it alone sets the stream time (76 us vs 60 us).
  - Rebalanced schedule: 17 rounds of up to 128 samples.  Rounds 0-12
    are full 128-row tiles; rounds 13-16 activate only contiguous
    partition ranges that EXCLUDE engine 15's partitions.  Row totals
    per engine: e15=104, others 124-132 -> stream ~62 us whether or not
    engine 15 is degraded.  Inactive (round, partition) slots get
    weight 0 (host writes idx=-1 -> is_equal misses) and the PE reads
    stale-but-finite data from 8 rounds earlier (slots 0-7 are first
    written by full rounds, so no uninitialized SBUF is ever read).
  - All compute is fp32-exact.  Per round the work is split so both
    engines stay under the ~3.4-3.8 us/round DMA pace:
      * TensorE, m-channels [0, MC): fused segment-sum on raw x,
          psum_big[g, (m,j)] += w[n, g]^T @ x[n, (m,j)]
      * VectorE, m-channels [MC, 64): spatial j-reduce to xs[n, m], then
        a small fp32 matmul accumulates psum_small[g, m] += w^T @ xs
    with w the scale-weighted one-hot (scale = 1/(count_g*49)), generated
    on device from a small aux tensor (iota/scale/per-round idx).
  - Epilogue (pipelined with the last round's matmul chunks): j-reduce
    psum_big -> out[:, :MC] in three m-chunks, copy psum_small ->
    out[:, MC:], and DMA out in four pieces as each chunk finishes.
    Host concatenates the 8 core results along m.

Raw Block implementation (not Tile): the walrus matmul/DMA lowerings only
accept ONE attached sync-wait per instruction; standalone wait_ge
instructions sidestep that.

DMA-completion semaphores: round r uses sem r % BUFS with a cumulative
threshold (16 per dma_start piece).  A shared counter is only safe
because a round's sem is reused (r+BUFS) strictly after round r was
consumed (the slot-reuse wait orders the re-issue).
"""

from contextlib import ExitStack

import numpy as np

import concourse.bass as bass
import concourse.mybir as mybir
from concourse.bass_utils import run_bass_kernel_spmd

N = 2048          # samples
M = 512           # channels
HW = 49           # spatial (7*7)
G = 64            # groups
CORES = 8
ML = M // CORES   # 64 channels per core
F = ML * HW       # 3136 floats per (n, core)
P = 128           # partitions per full tile
NT = 17           # rounds
BUFS = 8          # x-tile buffer depth == number of DMA semaphores

MC = 26           # m-channels handled by TensorE (raw fused matmul)
MV = ML - MC      # m-channels handled by VectorE reduce (38)
FC = MC * HW      # 1274 raw columns through the PE
# fp32 matmul chunks must stay within one 2KB PSUM bank -> 512-col chunks
CHUNKS = [(c * 512, min((c + 1) * 512, FC)) for c in range((FC + 511) // 512)]
NCH = len(CHUNKS)

# Active contiguous partition ranges per round.  Rounds 0-12 are full;
# rounds 13-16 exclude engine 15's partitions (92-95, 124-127) and are
# chosen so every other engine ends up with 124-132 rows total.
ACTIVE = {r: [(0, 128)] for r in range(13)}
ACTIVE[13] = [(0, 92)]
ACTIVE[14] = [(32, 92), (96, 124)]
ACTIVE[15] = [(0, 60), (96, 124)]
ACTIVE[16] = [(0, 88), (96, 124)]
ROWS = [sum(b - a for a, b in ACTIVE[r]) for r in range(NT)]
assert sum(ROWS) == N, sum(ROWS)

# cumulative dma-sem threshold for round r (sem r % BUFS, +16 per piece)
CUM = [0] * NT
for _r in range(NT):
    prev = CUM[_r - BUFS] if _r >= BUFS else 0
    CUM[_r] = prev + 16 * len(ACTIVE[_r])

# epilogue sub-reduce m-chunks of psum_big, aligned to the matmul chunks:
# sub-chunk i needs matmul chunks 0..need_i of the last round (pe_big
# counts one inc per chunk per round, in chunk order).
SUBRED = []
for _mlo, _mhi in ((0, 10), (10, 20), (20, MC)):
    _need = next(i for i, (lo, hi) in enumerate(CHUNKS) if hi >= _mhi * HW)
    SUBRED.append((_mlo, _mhi, (NT - 1) * NCH + _need + 1))
# output DMA pieces: 3 PE m-chunks + the vector-path tail, each issued as
# soon as its fin_sem increment lands.
OPIECES = [(0, 10), (10, 20), (20, MC), (MC, ML)]

F32 = mybir.dt.float32


def _build():
    nc = bass.Bass(trn_type="TRN2", target_bir_lowering=False)
    x_ext = nc.declare_dram_parameter("x", [N, F], F32, isOutput=False)
    # aux[:, 0:64] iota row, aux[:, 64:128] scale row, aux[:, 128:145] idx
    aux_ext = nc.declare_dram_parameter("aux", [P, G + G + NT], F32,
                                        isOutput=False)
    out_ext = nc.declare_dram_parameter("out", [G, ML], F32, isOutput=True)

    x_flat = x_ext.ap()  # [2048, 3136], rows pre-permuted into round order

    # HBM row offset of each round's first row
    roff = np.cumsum([0] + ROWS).tolist()

    with ExitStack() as ctx:
        x_buf = ctx.enter_context(nc.sbuf_tensor([P, BUFS * F], F32))
        xs_buf = ctx.enter_context(nc.sbuf_tensor([P, BUFS * MV], F32))
        aux_sb = ctx.enter_context(nc.sbuf_tensor([P, G + G + NT], F32))
        w_sb = ctx.enter_context(nc.sbuf_tensor([P, NT * G], F32))
        out_sb = ctx.enter_context(nc.sbuf_tensor([G, ML], F32))
        psum_big = ctx.enter_context(nc.psum_tensor([G, FC], F32))
        psum_small = ctx.enter_context(nc.psum_tensor([G, MV], F32))
        dma_x = [
            ctx.enter_context(nc.semaphore(name=f"dma_x{s}"))
            for s in range(BUFS)
        ]
        dma_a = ctx.enter_context(nc.semaphore())   # +16 when aux resident
        dma_o = ctx.enter_context(nc.semaphore())   # +16 per out piece
        wg_sem = ctx.enter_context(nc.semaphore())  # +1 when w generated
        red_sem = ctx.enter_context(nc.semaphore())  # +1 per round j-reduce
        pe_big = ctx.enter_context(nc.semaphore())   # +1 per big matmul chunk
        pe_tile = ctx.enter_context(nc.semaphore())  # +1 per round (small mm)
        fin_sem = ctx.enter_context(nc.semaphore())  # +1 per out_sb piece
        block = ctx.enter_context(nc.Block())

        def xwait(engine, r):
            engine.wait_ge(dma_x[r % BUFS], CUM[r])

        # ---- DMA program (SP / HWDGE, FIFO) ----
        @block.sync
        def _(sync):
            def xdma(r):
                if r >= BUFS:
                    # slot reuse: the small matmul is ordered after both the
                    # j-reduce and the big matmuls of its round
                    sync.wait_ge(pe_tile, r - BUFS + 1)
                slot = r % BUFS
                off = roff[r]
                for a, b in ACTIVE[r]:
                    sync.dma_start(
                        out=x_buf[a:b, slot * F:(slot + 1) * F],
                        in_=x_flat[off:off + (b - a), :],
                    ).then_inc(dma_x[slot], 16)
                    off += b - a

            xdma(0)
            sync.dma_start(out=aux_sb[:, :], in_=aux_ext.ap()).then_inc(dma_a, 16)
            for r in range(1, NT):
                xdma(r)
            for i, (lo, hi) in enumerate(OPIECES):
                sync.wait_ge(fin_sem, i + 1)
                sync.dma_start(
                    out=out_ext.ap()[:, lo:hi], in_=out_sb[:, lo:hi]
                ).then_inc(dma_o, 16)
            sync.wait_ge(dma_o, 16 * len(OPIECES))

        # ---- VectorE: w generation, j-reduction, epilogue ----
        @block.vector
        def _(vector):
            # generate the scale-weighted one-hot from idx:
            #   w[p, r*G+g] = (idx[r, p] == g) * scale[g]   (idx=-1 -> 0)
            vector.wait_ge(dma_a, 16)
            for r in range(NT):
                wg = vector.scalar_tensor_tensor(
                    out=w_sb[:, r * G:(r + 1) * G],
                    in0=aux_sb[:, 0:G],
                    scalar=aux_sb[:, 2 * G + r:2 * G + r + 1],
                    in1=aux_sb[:, G:2 * G],
                    op0=mybir.AluOpType.is_equal,
                    op1=mybir.AluOpType.mult,
                )
            wg.then_inc(wg_sem, 1)

            for r in range(NT):
                xwait(vector, r)
                if r >= BUFS:
                    # xs slot reuse: wait until round r-BUFS consumed by PE
                    vector.wait_ge(pe_tile, r - BUFS + 1)
                slot = r % BUFS
                vector.tensor_reduce(
                    out=xs_buf[:, slot * MV:(slot + 1) * MV],
                    in_=x_buf[:, slot * F + FC:(slot + 1) * F].rearrange(
                        "p (m j) -> p m j", j=HW
                    ),
                    axis=mybir.AxisListType.X,
                    op=mybir.AluOpType.add,
                ).then_inc(red_sem, 1)

            # epilogue: j-reduce psum_big in m-chunks as the last round's
            # matmul chunks complete; copy psum_small
            for mlo, mhi, need in SUBRED:
                vector.wait_ge(pe_big, need)
                vector.tensor_reduce(
                    out=out_sb[:, mlo:mhi],
                    in_=psum_big[:, mlo * HW:mhi * HW].rearrange(
                        "p (m j) -> p m j", j=HW
                    ),
                    axis=mybir.AxisListType.X,
                    op=mybir.AluOpType.add,
                ).then_inc(fin_sem, 1)
            vector.wait_ge(pe_tile, NT)
            vector.tensor_copy(
                out_sb[:, MC:ML], psum_small[:, :]
            ).then_inc(fin_sem, 1)

        # ---- TensorE: segment-sum accumulation (fp32) ----
        @block.tensor
        def _(tensor):
            tensor.wait_ge(wg_sem, 1)
            for r in range(NT):
                xwait(tensor, r)
                slot = r % BUFS
                wt = w_sb[:, r * G:(r + 1) * G]
                for lo, hi in CHUNKS:
                    tensor.matmul(
                        out=psum_big[:, lo:hi],
                        lhsT=wt,
                        rhs=x_buf[:, slot * F + lo:slot * F + hi],
                        start=(r == 0),
                        stop=(r == NT - 1),
                    ).then_inc(pe_big, 1)
                tensor.wait_ge(red_sem, r + 1)
                tensor.matmul(
                    out=psum_small[:, :],
                    lhsT=wt,
                    rhs=xs_buf[:, slot * MV:(slot + 1) * MV],
                    start=(r == 0),
                    stop=(r == NT - 1),
                ).then_inc(pe_tile, 1)

    return nc


def _prepare(x, idx):
    x = np.asarray(x)
    if x.dtype != np.float32:
        x = x.astype(np.float32)
    idx = np.asarray(idx).astype(np.int64)
    counts = np.bincount(idx, minlength=G).astype(np.float64)
    scale = np.where(counts > 0, 1.0 / (counts * HW), 0.0).astype(np.float32)

    # samples fill (round, active-partition) slots in natural order, so x
    # rows stay unpermuted; only the per-slot idx placement encodes the
    # schedule.  Inactive slots keep idx=-1 -> w=0.
    aux = np.zeros((P, G + G + NT), np.float32)
    aux[:, 0:G] = np.arange(G, dtype=np.float32)[None, :]
    aux[:, G:2 * G] = scale[None, :]
    aux[:, 2 * G:] = -1.0
    n = 0
    for r in range(NT):
        for a, b in ACTIVE[r]:
            cnt = b - a
            aux[a:b, 2 * G + r] = idx[n:n + cnt].astype(np.float32)
            n += cnt
    assert n == N

    xr = x.reshape(N, M, HW)
    in_maps = []
    for k in range(CORES):
        shard = np.ascontiguousarray(xr[:, k * ML:(k + 1) * ML, :]).reshape(N, F)
        in_maps.append({"x": shard, "aux": aux})
    return in_maps


def run(x, tensor_list_assignmentindices, trace=False):
    in_maps = _prepare(x, tensor_list_assignmentindices)
    nc = _build()
    res = run_bass_kernel_spmd(nc, in_maps, core_ids=list(range(CORES)), trace=trace)
    outs = [np.asarray(r["out"]) for r in res.results]
    out = np.concatenate(outs, axis=1)  # [G, M]
    return out.reshape(G, M, 1, 1).astype(np.float32), res.exec_time_ns


def kernel(**inputs):
    out, _ = run(inputs["x"], inputs["tensor_list_assignmentindices"], trace=False)
    return out
